# revision 1
# baseline (speedup 1.0000x reference)
"""Trainium2 Bass kernel for per-pixel local convolution (LocalConvolution).

Reference semantics (fp32):
    out[n, g*8+j, ho, wo] = sum_{i,jj in 5x5} x[n, g*8+j, ho+i-2, wo+jj-2]
                                             * w[n, j, i*5+jj, ho*128+wo]
with zero padding, N=4, C=64, H=W=128, CW=8, K=5.

Sharding: 8 cores = (batch n in 4) x (H-half in 2). Each core computes a
[64, 64, 128] output slab from a host-padded input slab [64, 69, 132] and a
weight slab [8, 25, 8192].

On-core layout: 128 partitions = (weight-channel j in 8) x (4-row block pc in
16); looping over the 8 channel groups g reuses one weight residency (no
replication). Pixels are streamed as 528 "pseudo-pixels" per g and partition
(4 rows x 132 cols incl. right-pad); pad columns carry zero weights so they
contribute nothing, which keeps every tap stream a contiguous slice.

Compute paths:
  LC_PHASE=1  stock two-pass: 25x tensor_mul + strided tensor_reduce over taps
  LC_PHASE=2  (default) fused single-pass: custom DVE op out = cumsum(in0*in1)
              along the free stream (1 MAC/cycle/lane, the fp32 2-read-port
              bound); per-pixel sums are recovered on GPSIMD (overlapped) by
              sampling the prefix at tap boundaries and differencing.
"""

import os

import numpy as np

try:
    import concourse.bass as bass
except ImportError:  # fresh grading dir: concourse lives in the container image
    import sys

    for p in ("/opt/trn_rl_repo", "/root/.axon_site/_ro/trn_rl_repo"):
        if p not in sys.path:
            sys.path.insert(0, p)
    import concourse.bass as bass

import concourse.mybir as mybir
from concourse import tile
from concourse.bass_utils import run_bass_kernel_spmd

# ---------------------------------------------------------------------------
# Workaround: this walrus build accepts only ONE sync-wait per instruction,
# but Tile's semaphore assignment freely fuses several. Post-pass: peel extra
# waits off onto preceding same-engine NOPs (engine streams execute in order,
# so the NOPs block the engine until every condition holds).
# ---------------------------------------------------------------------------


def _split_multi_waits(nc):
    n_split = 0
    for fn in nc.m.functions:
        for bb in fn.blocks:
            new_insts = []
            for inst in bb.instructions:
                si = inst.sync_info
                if si is not None and len(si.on_wait) > 1:
                    waits = list(si.on_wait)
                    for k, w in enumerate(waits[:-1]):
                        n_split += 1
                        new_insts.append(
                            mybir.InstNoOp(
                                name=f"{inst.name}_w{k}",
                                engine=inst.engine,
                                sync_info=mybir.SyncInfo(
                                    on_wait=[w], on_update=[]
                                ),
                                bass_nofuse=True,
                            )
                        )
                    inst.sync_info = mybir.SyncInfo(
                        on_wait=[waits[-1]], on_update=list(si.on_update)
                    )
                new_insts.append(inst)
            bb.instructions = new_insts
    return n_split

# ---------------------------------------------------------------------------
# Custom DVE op: fused multiply + running prefix sum along the free stream.
#   out[p, t] = sum_{t' <= t} in0[p, t'] * in1[p, t']      (fp32 accumulator)
# ---------------------------------------------------------------------------


def _register_mac_prefix():
    from concourse import dve_ops
    from concourse.dve_spec import (
        AluOp,
        Spec,
        Src0,
        Src1,
        _has_src1,
        lower,
        scan,
    )
    from concourse.dve_table_gen import dve_ver_for
    from concourse.dve_uop import DveOpSpec

    name = "MAC_PREFIX_ANT"
    if name in dve_ops._SUB_OPCODE_FOR_NAME:
        return next(op for op in dve_ops.OPS if op.name == name)

    def _ref(in0, in1, s0, s1, imm2):
        prod = in0.astype(np.float32) * in1.astype(np.float32)
        flat = prod.reshape(prod.shape[0], -1)
        return np.cumsum(flat, axis=1).reshape(prod.shape)

    spec = Spec(body=scan(AluOp.ADD, Src0 * Src1), reference=_ref)
    row = dve_ops._CUSTOM_DVE_ROW_BASE + len(dve_ops.OPS)
    assert row < 0x20
    shas = {}
    for ver in {dve_ver_for("TRN2"), "v3", "v4"}:
        compiled = DveOpSpec(
            name=name, opcode=row, uops=lower(spec, ver=ver), rd1_en=_has_src1(spec)
        )
        shas[ver] = compiled.sha(ver)
    op = dve_ops.DveOp(name, spec, subdim=False, uops_sha=shas)
    dve_ops.OPS.append(op)
    dve_ops.CUSTOM_DVE_SPECS[name] = spec
    dve_ops._SUB_OPCODE_FOR_NAME[name] = row
    return op


# ---------------------------------------------------------------------------
# Problem constants
# ---------------------------------------------------------------------------
N, C, H, W = 4, 64, 128, 128
K, PAD, CW = 5, 2, 8
HO, WO = 128, 128
RH = 64  # output rows per core
WP = W + 2 * PAD  # 132: padded row length
HP = RH + K  # 69: padded rows per core slab (64 + 4 halo + 1 guard)
NJ, NPC, RB = 8, 16, 4  # partition = j*16 + pc; RB output rows per pc
Q = RB * WP  # 528 pseudo-pixels per partition per group
F32 = mybir.dt.float32
X = mybir.AxisListType.X
ADD = mybir.AluOpType.add

PHASE = int(os.environ.get("LC_PHASE", "2"))


def _build_program(phase, repeat=1):
    nc = bass.Bass()
    xpad_d = nc.declare_dram_parameter("xpad", [C, HP, WP], F32, isOutput=False)
    w_d = nc.declare_dram_parameter("w", [CW, K * K, RH * WO], F32, isOutput=False)
    out_d = nc.declare_dram_parameter("out", [C, RH, WO], F32, isOutput=True)

    if phase == 2:
        mac_prefix = _register_mac_prefix()

    xpad_a = xpad_d[:]
    w_a = w_d[:]
    out_a = out_d[:]

    with tile.TileContext(nc) as tc:
        with (
            tc.tile_pool(name="wpool", bufs=1) as wpool,
            tc.tile_pool(name="xpool", bufs=3) as xpool,
            tc.tile_pool(name="big", bufs=2) as bigpool,
            tc.tile_pool(name="tpool", bufs=2) as tpool,
            tc.tile_pool(name="ogpool", bufs=3) as ogpool,
        ):
            # ---- weight residency: 5 tiles (one per kernel row i), layout
            # [p=(j,pc), jj, q] with q = 4x132 pseudo-pixels, cols 128..131
            # zeroed so pad pixels multiply to exactly 0.
            w_tiles = []
            for i in range(K):
                wt = wpool.tile([128, K * Q], F32, tag=f"w{i}")
                wa = wt[:]
                # zero the 4 pad columns of each (jj, row) stripe
                pad_ap = wa.__replace__(
                    ap=[wa.ap[0], [Q, K], [WP, RB], [1, WP - WO]],
                    offset=wa.offset + WO,
                )
                nc.gpsimd.memset(pad_ap, 0.0)
                for jj in range(K):
                    kk = i * K + jj
                    dst = wa.__replace__(
                        ap=[wa.ap[0], [WP, RB], [1, WO]],
                        offset=wa.offset + jj * Q,
                    )
                    src = w_a.__replace__(
                        ap=[[K * K * RH * WO, NJ], [RB * WO, NPC], [1, RB * WO]],
                        offset=kk * RH * WO,
                    )
                    nc.sync.dma_start(dst, src)
                w_tiles.append(wt)

            for g in range(repeat * C // CW):
                g = g % (C // CW)
                # ---- input slab for this channel group: partition (j, pc)
                # holds 9 padded rows x 132 cols of channel c = g*8+j.
                xg = xpool.tile([128, (RB + K) * WP], F32, tag="xg")
                xa = xg[:]
                nc.sync.dma_start(
                    xa,
                    xpad_a.__replace__(
                        ap=[[HP * WP, NJ], [RB * WP, NPC], [1, (RB + K) * WP]],
                        offset=g * CW * HP * WP,
                    ),
                )

                og = ogpool.tile([128, Q], F32, tag="og")
                if phase == 1:
                    prod = bigpool.tile([128, K * K * Q], F32, tag="prod")
                    for kk in range(K * K):
                        i, jj = divmod(kk, K)
                        nc.vector.tensor_mul(
                            prod[:, kk * Q : (kk + 1) * Q],
                            xg[:, i * WP + jj : i * WP + jj + Q],
                            w_tiles[i][:, jj * Q : (jj + 1) * Q],
                        )
                    pa = prod[:]
                    nc.vector.tensor_reduce(
                        og[:, :],
                        pa.__replace__(ap=[pa.ap[0], [1, Q], [Q, K * K]]),
                        axis=X,
                        op=ADD,
                    )
                else:
                    # T[q] accumulates each kernel row's prefix sampled at its
                    # tap boundary (jj=4)
                    t = tpool.tile([128, Q], F32, tag="t")
                    for i in range(K):
                        pre = bigpool.tile([128, K * Q], F32, tag="pre")
                        prea = pre[:]
                        in0 = xa.__replace__(
                            ap=[xa.ap[0], [1, Q], [1, K]],
                            offset=xa.offset + i * WP,
                        )
                        wa = w_tiles[i][:]
                        in1 = wa.__replace__(
                            ap=[wa.ap[0], [1, Q], [Q, K]], offset=wa.offset
                        )
                        nc.vector._custom_dve(
                            mac_prefix, out=prea, in0=in0, in1=in1
                        )
                        boundary = prea.__replace__(
                            ap=[prea.ap[0], [K, Q]], offset=prea.offset + (K - 1)
                        )
                        # boundary extraction runs on GPSIMD, overlapping the
                        # next scan on the vector engine
                        if i == 0:
                            nc.gpsimd.tensor_copy(t[:, :], boundary)
                        else:
                            nc.gpsimd.tensor_add(t[:, :], t[:, :], boundary)
                    # per-pixel sums: out[q] = T[q] - T[q-1] (garbage pixels
                    # contribute zero, so row-crossing diffs stay exact)
                    nc.gpsimd.tensor_sub(og[:, 1:Q], t[:, 1:Q], t[:, 0 : Q - 1])
                    nc.gpsimd.tensor_copy(og[:, 0:1], t[:, 0:1])

                oga = og[:]
                nc.sync.dma_start(
                    out_a.__replace__(
                        ap=[[RH * WO, NJ], [RB * WO, NPC], [WO, RB], [1, WO]],
                        offset=g * CW * RH * WO,
                    ),
                    oga.__replace__(
                        ap=[oga.ap[0], [WP, RB], [1, WO]], offset=oga.offset
                    ),
                )
    # raw Bass skips the ISA-subclass byte encoding pass that Bacc.compile
    # runs; without it the NEFF compiler sees empty .instr -> "ISA wrong length"
    mybir.codegen_inst_isa_subclasses(nc)
    _split_multi_waits(nc)
    return nc


def _shard_inputs(input, weight):
    input = np.asarray(input, dtype=np.float32)
    weight = np.asarray(weight, dtype=np.float32)
    in_maps = []
    for n in range(N):
        xp = np.pad(input[n], ((0, 0), (PAD, PAD + 1), (PAD, PAD)))  # [64,133,132]
        for half in range(2):
            r0 = RH * half
            in_maps.append(
                {
                    "xpad": np.ascontiguousarray(xp[:, r0 : r0 + HP, :]),
                    "w": np.ascontiguousarray(
                        weight[n, :, :, r0 * WO : (r0 + RH) * WO]
                    ),
                }
            )
    return in_maps


def kernel(input, weight):
    nc = _build_program(PHASE)
    in_maps = _shard_inputs(input, weight)
    res = run_bass_kernel_spmd(nc, in_maps, list(range(8)))
    out = np.empty((N, C, HO, WO), dtype=np.float32)
    for k in range(8):
        n, half = divmod(k, 2)
        out[n, :, RH * half : RH * (half + 1), :] = res.results[k]["out"]
    return out



# revision 37
# speedup vs baseline: 1.3462x; 1.3462x over previous
"""Trainium2 Bass kernel for per-pixel local convolution (LocalConvolution).

Reference semantics (fp32):
    out[n, g*8+j, ho, wo] = sum_{i,jj in 5x5} x[n, g*8+j, ho+i-2, wo+jj-2]
                                             * w[n, j, i*5+jj, ho*128+wo]
with zero padding, N=4, C=64, H=W=128, CW=8, K=5.

Sharding: 8 cores = (batch n in 4) x (H-half in 2). Each core computes a
[64, 64, 128] output slab from a host-padded fp16 input slab [64, 69, 132]
and a host-prepacked pixel-major fp16 weight slab [128, 528*25].

On-core layout: 128 partitions = (weight-channel j in 8) x (4-row block pc
in 16); looping over the 8 channel groups g reuses one weight residency.
Pixels are streamed as Q=528 "pseudo-pixels" per partition per group (4 rows
x 132 cols incl. right-pad); pad columns carry zero weights so they
contribute nothing, keeping every tap stream a contiguous slice.

Compute (per group): one DVE fp16 tensor_mul at the 2x perf mode (0.53
ns/elem) produces the pixel-major product stream prod[q*25+kk]; a 5-level
in-place pairwise tree of stock TensorTensor adds then reduces the 25 taps
per pixel. Levels L1 (9 elems/pixel) and L5 (the final pair -> og) run on
the otherwise-idle Pool/GPSIMD engine, L2-L4 on DVE with packed inner dims
(2x mode), balancing the two engines at ~87us of busy work each. A 3-stage
software pipeline (mul_{g+1} | L1_g | L2-4_{g-1} | L5_{g-2}+DMA) keeps every
cross-engine dependency one full group old.
"""

import os

import numpy as np

try:
    import concourse.bass as bass
except ImportError:  # fresh grading dir: concourse lives in the container image
    import sys

    for p in ("/opt/trn_rl_repo", "/root/.axon_site/_ro/trn_rl_repo"):
        if p not in sys.path:
            sys.path.insert(0, p)
    import concourse.bass as bass

import concourse.mybir as mybir
from concourse import tile
from concourse.bass_utils import run_bass_kernel_spmd

# ---------------------------------------------------------------------------
# Workaround: this walrus build accepts only ONE sync-wait per instruction,
# but Tile's semaphore assignment freely fuses several. Post-pass: peel extra
# waits off onto preceding same-engine NOPs (engine streams execute in order,
# so the NOPs block the engine until every condition holds).
# ---------------------------------------------------------------------------


def _split_multi_waits(nc):
    n_split = 0
    for fn in nc.m.functions:
        for bb in fn.blocks:
            new_insts = []
            for inst in bb.instructions:
                si = inst.sync_info
                if si is not None and len(si.on_wait) > 1:
                    waits = list(si.on_wait)
                    for k, w in enumerate(waits[:-1]):
                        n_split += 1
                        new_insts.append(
                            mybir.InstNoOp(
                                name=f"{inst.name}_w{k}",
                                engine=inst.engine,
                                sync_info=mybir.SyncInfo(
                                    on_wait=[w], on_update=[]
                                ),
                                bass_nofuse=True,
                            )
                        )
                    inst.sync_info = mybir.SyncInfo(
                        on_wait=[waits[-1]], on_update=list(si.on_update)
                    )
                new_insts.append(inst)
            bb.instructions = new_insts
    return n_split

# ---------------------------------------------------------------------------
# Problem constants
# ---------------------------------------------------------------------------
N, C, H, W = 4, 64, 128, 128
K, PAD, CW = 5, 2, 8
KK = K * K
HO, WO = 128, 128
RH = 64  # output rows per core
WP = W + 2 * PAD  # 132: padded row length
HP = RH + K  # 69: padded rows per core slab (64 + 4 halo + 1 guard)
NJ, NPC, RB = 8, 16, 4  # partition = j*16 + pc; RB output rows per pc
Q = RB * WP  # 528 pseudo-pixels per partition per group
F32 = mybir.dt.float32
F16 = mybir.dt.float16
ADD = mybir.AluOpType.add
MULT = mybir.AluOpType.mult



def _build_program(qa=None, qa0=None):
    """v3: per group, one DVE fp16 2x multiply produces the pixel-major
    product stream prod[q*25+kk]; a 5-level in-place pairwise tree then
    reduces the 25 taps per pixel:
        L1: prod[0:9)  += prod[16:25)   (9 elems/pixel)
        L2: prod[0:8)  += prod[8:16)    (8)
        L3: prod[0:4)  += prod[4:8)     (4)
        L4: prod[0:2)  += prod[2:4)     (2)
        L5: og[q] = prod[25q] + prod[25q+1]
    All levels are stock TensorTensor ops with packed inner dims, so they
    run on either engine; L1 + L5 + a pixel-slice of L2 go to Pool, the
    rest (and the mul) to DVE, balancing the two engines. The software
    pipeline keeps every cross-engine dependency one full group old.
    """
    nc = bass.Bass()
    xpad_d = nc.declare_dram_parameter("xpad", [C, HP, WP], F16, isOutput=False)
    w_d = nc.declare_dram_parameter("w", [128, KK * Q], F16, isOutput=False)
    out_d = nc.declare_dram_parameter("out", [C, RH, WO], F16, isOutput=True)

    xpad_a = xpad_d[:]
    w_a = w_d[:]
    out_a = out_d[:]

    NG = C // CW
    QL2 = int(os.environ.get("LC_QL2", "0"))  # pixels of L2 done on Pool

    with tile.TileContext(nc) as tc:
        with (
            tc.tile_pool(name="wpool", bufs=1) as wpool,
            tc.tile_pool(name="xpool", bufs=4) as xpool,
            tc.tile_pool(name="prodp", bufs=5) as prodp,
            tc.tile_pool(name="ogpool", bufs=3) as ogpool,
        ):
            def x_dma(g):
                xg = xpool.tile([128, (RB + K) * WP], F16, tag="xg")
                nc.sync.dma_start(
                    xg[:],
                    xpad_a.__replace__(
                        ap=[[HP * WP, NJ], [RB * WP, NPC], [1, (RB + K) * WP]],
                        offset=g * CW * HP * WP,
                    ),
                )
                return xg

            xgs = {0: x_dma(0), 1: x_dma(1)}

            # weight residency (pixel-major, host-prepacked, pad cols zero);
            # chunked on the Act HWDGE queue so group 0's first mul chunk
            # can start as soon as its bytes land.
            wt = wpool.tile([128, KK * Q], F16, tag="w")
            wa = wt[:]
            for lo, hi in ((0, 66), (66, 198), (198, 396), (396, Q)):
                nc.scalar.dma_start(
                    wt[:, lo * KK : hi * KK], w_a[:, lo * KK : hi * KK]
                )

            prods, ogs = {}, {}

            def mul_chunk(g, q0, q1):
                xa = xgs[g][:]
                proda = prods[g][:]
                nc.vector.tensor_tensor(
                    proda.__replace__(
                        ap=[proda.ap[0], [KK, q1 - q0], [K, K], [1, K]],
                        offset=proda.offset + q0 * KK,
                    ),
                    xa.__replace__(
                        ap=[xa.ap[0], [1, q1 - q0], [WP, K], [1, K]],
                        offset=xa.offset + q0,
                    ),
                    wa.__replace__(
                        ap=[wa.ap[0], [KK, q1 - q0], [K, K], [1, K]],
                        offset=wa.offset + q0 * KK,
                    ),
                    op=MULT,
                )

            def emit_mul(g, chunks=((0, Q),)):
                prod_t = prodp.tile([128, KK * Q], F16, tag="prod")
                prods[g] = prod_t
                for q0, q1 in chunks:
                    mul_chunk(g, q0, q1)

            def tree_level(eng, g, dst_off, src_off, cnt, q0, q1):
                pa_ = prods[g][:]
                dst = pa_.__replace__(
                    ap=[pa_.ap[0], [KK, q1 - q0], [1, cnt]],
                    offset=pa_.offset + q0 * KK + dst_off,
                )
                src = pa_.__replace__(
                    ap=[pa_.ap[0], [KK, q1 - q0], [1, cnt]],
                    offset=pa_.offset + q0 * KK + src_off,
                )
                eng.tensor_add(dst, dst, src)

            def emit_L1(g, q0=0, q1=Q):
                tree_level(nc.gpsimd, g, 0, 16, 9, q0, q1)

            def emit_L2p(g):
                if QL2 > 0:
                    tree_level(nc.gpsimd, g, 0, 8, 8, 0, QL2)

            def emit_L24(g, q0=None, q1=Q):
                if q0 is None:
                    tree_level(nc.vector, g, 0, 8, 8, QL2, q1)
                else:
                    tree_level(nc.vector, g, 0, 8, 8, q0, q1)
                    q0 = q0
                q0 = 0 if q0 is None else q0
                tree_level(nc.vector, g, 0, 4, 4, q0, q1)
                tree_level(nc.vector, g, 0, 2, 2, q0, q1)

            def emit_L5(g, q0=0, q1=Q):
                if g not in ogs:
                    og_t = ogpool.tile([128, Q], F16, tag="og")
                    ogs[g] = og_t
                og = ogs[g]
                pa_ = prods[g][:]
                oga = og[:]
                nc.gpsimd.tensor_add(
                    oga.__replace__(
                        ap=[oga.ap[0], [1, q1 - q0]], offset=oga.offset + q0
                    ),
                    pa_.__replace__(
                        ap=[pa_.ap[0], [KK, q1 - q0]], offset=pa_.offset + q0 * KK
                    ),
                    pa_.__replace__(
                        ap=[pa_.ap[0], [KK, q1 - q0]],
                        offset=pa_.offset + q0 * KK + 1,
                    ),
                )

            def emit_out(g, rows=(0, RB)):
                oga = ogs[g][:]
                r0, r1 = rows
                nc.sync.dma_start(
                    out_a.__replace__(
                        ap=[[RH * WO, NJ], [RB * WO, NPC], [WO, r1 - r0], [1, WO]],
                        offset=g * CW * RH * WO + r0 * WO,
                    ),
                    oga.__replace__(
                        ap=[oga.ap[0], [WP, r1 - r0], [1, WO]],
                        offset=oga.offset + r0 * WP,
                    ),
                )

            # --- software pipeline: stage s handles mul_{g+1}, L1_g,
            # L2p_{g-1}, L2-4_{g-1} (DVE), L5_{g-2} + its output DMA.
            G0_CH = ((0, 66), (66, 198), (198, 396), (396, Q))
            emit_mul(0, chunks=G0_CH)

            for g in range(NG):
                # Pool: L1 of this group (its mul finished last stage)
                if g == 0:
                    for q0, q1 in G0_CH:
                        emit_L1(0, q0, q1)
                else:
                    emit_L1(g)

                if g + 2 < NG:
                    xgs[g + 2] = x_dma(g + 2)
                if g + 1 < NG:
                    emit_mul(g + 1)

                if g >= 1:
                    emit_L2p(g - 1)      # Pool slice of L2
                    if g == 1:
                        for q0, q1 in G0_CH:
                            emit_L24(0, max(q0, QL2), q1)
                    else:
                        emit_L24(g - 1)  # DVE: rest of L2, L3, L4
                if g >= 2:
                    emit_L5(g - 2)       # Pool: final pairs -> og
                    emit_out(g - 2)

            # pipeline flush: last group's tree in halves so the final
            # L5+DMA chain overlaps the first half's work
            emit_L2p(NG - 1)
            emit_L24(NG - 1, max(QL2, 0), 264)
            emit_L5(NG - 2)
            emit_out(NG - 2)
            emit_L5(NG - 1, 0, 264)
            emit_out(NG - 1, rows=(0, 2))
            emit_L24(NG - 1, 264, Q)
            emit_L5(NG - 1, 264, Q)
            emit_out(NG - 1, rows=(2, RB))
    # raw Bass skips the ISA-subclass byte encoding pass that Bacc.compile
    # runs; without it the NEFF compiler sees empty .instr -> "ISA wrong length"
    mybir.codegen_inst_isa_subclasses(nc)
    _split_multi_waits(nc)
    return nc


def _shard_inputs(input, weight):
    input = np.asarray(input, dtype=np.float32)
    weight = np.asarray(weight, dtype=np.float32)
    in_maps = []
    for n in range(N):
        xp = np.pad(input[n], ((0, 0), (PAD, PAD + 1), (PAD, PAD)))  # [64,133,132]
        # weight -> per-partition pixel-major [j*16+pc, q*25+kk] with the 4
        # right-pad columns of each row zeroed.
        wn = weight[n].reshape(CW, KK, HO, WO)  # [8, 25, 128, 128]
        wn = np.pad(wn, ((0, 0), (0, 0), (0, 0), (0, WP - WO)))  # [8,25,128,132]
        for half in range(2):
            r0 = RH * half
            ws = wn[:, :, r0 : r0 + RH, :]  # [8, 25, 64, 132]
            ws = ws.reshape(CW, KK, NPC, RB * WP)  # [8, 25, 16, 528]
            ws = ws.transpose(0, 2, 3, 1)  # [8, 16, 528, 25]
            in_maps.append(
                {
                    "xpad": np.ascontiguousarray(
                        xp[:, r0 : r0 + HP, :], dtype=np.float16
                    ),
                    "w": np.ascontiguousarray(
                        ws.reshape(128, KK * Q), dtype=np.float16
                    ),
                }
            )
    return in_maps


def kernel(input, weight):
    nc = _build_program()
    in_maps = _shard_inputs(input, weight)
    res = run_bass_kernel_spmd(nc, in_maps, list(range(8)))
    out = np.empty((N, C, HO, WO), dtype=np.float32)
    for k in range(8):
        n, half = divmod(k, 2)
        out[n, :, RH * half : RH * (half + 1), :] = res.results[k][
            "out"
        ].astype(np.float32)
    return out
